# revision 10
# baseline (speedup 1.0000x reference)
"""Trainium2 Bass kernel for nn_BatchMAFLayer.

Computes, for a batch of B=4096 inputs and N_BATCH=64 MADE/MAF
distributions, the per-(sample, distribution) log-likelihood:

    xr = x[:, :32]
    h  = relu(xr @ (w1*m1)[n]); h = relu(h @ (w2*m2)[n]); o = h @ (w3*m3)[n]
    shift, ls = o d-major-deinterleaved
    y  = (xr - shift) * exp(-ls)
    ll[b, n] = sum_d(-0.5*y^2 - C - ls)

Sharding: the 64 MADEs are split across 8 NeuronCores (8 per core);
every core sees the full batch and writes its 8 output columns.

On-device layout is feature-major ("T-space"): activations are stored
as [features, batch] so each layer is matmul(out=W.T @ actT) with the
weight stack used directly as the stationary (lhsT) operand and no
transposes between layers. The final d-reduction is one more matmul
against a constant coefficient vector.
"""

import numpy as np

import concourse.bass as bass
from concourse import bacc
import concourse.mybir as mybir
import concourse.tile as tile

F32 = mybir.dt.float32
F32R = mybir.dt.float32r
U8 = mybir.dt.uint8
AFT = mybir.ActivationFunctionType

D = 32
N_BATCH = 64
HID = 256
B = 4096
F = 64
NCORES = 8
NPC = N_BATCH // NCORES  # mades per core
CH = 512                 # batch columns per PSUM tile
NCH = B // CH
HALF_LOG_2PI = 0.5 * float(np.log(2.0 * np.pi))

# matmul input dtype: float32r streams at bf16 rate (1 col/cycle) for
# free-dim >= 256 while plain float32 runs at 1/4 rate.
MM_DT = F32R


def _mm(ap):
    return ap


def build_nc():
    nc = bacc.Bacc("TRN2", target_bir_lowering=False)

    xT_d = nc.dram_tensor("xT", [D, B], F32R, kind="ExternalInput")
    w1_d = nc.dram_tensor("w1", [NPC, D, HID], F32R, kind="ExternalInput")
    w2_d = nc.dram_tensor("w2", [NPC, HID, HID], F32R, kind="ExternalInput")
    w3_d = nc.dram_tensor("w3", [NPC, HID, 2 * D], F32R, kind="ExternalInput")
    m1_d = nc.dram_tensor("m1", [NPC, D, HID], U8, kind="ExternalInput")
    m2_d = nc.dram_tensor("m2", [NPC, HID, HID], U8, kind="ExternalInput")
    m3_d = nc.dram_tensor("m3", [NPC, HID, 2 * D], U8, kind="ExternalInput")
    out_d = nc.dram_tensor("out", [NPC, B], F32, kind="ExternalOutput")

    with tile.TileContext(nc) as tc:
        with (
            tc.tile_pool(name="persist", bufs=1) as persist,
            tc.tile_pool(name="stage", bufs=2) as stage,
            tc.tile_pool(name="work", bufs=2) as work,
            tc.tile_pool(name="psum_h", bufs=4, space="PSUM") as psum_h,
            tc.tile_pool(name="psum_o", bufs=2, space="PSUM") as psum_o,
            tc.tile_pool(name="psum_l", bufs=2, space="PSUM") as psum_l,
        ):
            # x[:, :D] transposed, replicated to all four 32-partition
            # groups so small-K matmuls can row-tile the PE array.
            xrT = persist.tile([128, B], F32R, tag="xrT")
            for r in range(4):
                nc.sync.dma_start(out=xrT[32 * r : 32 * r + 32, :], in_=xT_d[:, :])

            # coefficients for the final d-reduction matmul:
            # ll = sum_d(-0.5 * y_d^2) + sum_d(-(ls_d + C))
            coeff_f = persist.tile([64, 1], F32, tag="coeff_f")
            nc.vector.memset(coeff_f[0:32, :], -0.5)
            nc.vector.memset(coeff_f[32:64, :], -1.0)
            coeff = persist.tile([64, 1], F32R, tag="coeff")
            nc.vector.tensor_copy(coeff, coeff_f)

            # per-partition bias columns for ScalarE activations
            zbias = persist.tile([128, 1], F32, tag="zbias")
            nc.vector.memset(zbias, 0.0)
            cbias = persist.tile([32, 1], F32, tag="cbias")
            nc.vector.memset(cbias, HALF_LOG_2PI)

            # ---- weight prep -------------------------------------------------
            # W1 packed 4 mades per [128, HID] tile (made i at partitions 32i).
            W1 = []
            for g in range(NPC // 4):
                w1t = persist.tile([128, HID], F32R, tag=f"w1_{g}")
                nc.sync.dma_start(
                    out=w1t, in_=w1_d[4 * g : 4 * g + 4].rearrange("a p f -> (a p) f")
                )
                m1t = stage.tile([128, HID], U8, tag="m1s")
                nc.sync.dma_start(
                    out=m1t, in_=m1_d[4 * g : 4 * g + 4].rearrange("a p f -> (a p) f")
                )
                m1f = stage.tile([128, HID], F32R, tag="m1f")
                nc.vector.tensor_copy(m1f, m1t)
                nc.vector.tensor_mul(w1t, w1t, m1f)
                W1.append(w1t)

            # W2 per made: [128, 2*HID]; cols [0:HID] = rows 0:128 (k-half a),
            # cols [HID:] = rows 128:256 (k-half b).
            W2 = []
            M2v = m2_d.rearrange("n (a p) f -> n p a f", a=2)
            for n in range(NPC):
                w2t = persist.tile([128, 2 * HID], F32R, tag=f"w2_{n}")
                nc.sync.dma_start(
                    out=w2t.rearrange("p (a f) -> p a f", a=2),
                    in_=w2_d[n].rearrange("(a p) f -> p a f", a=2),
                )
                m2t = stage.tile([128, 2 * HID], U8, tag="m2s")
                nc.sync.dma_start(
                    out=m2t.rearrange("p (a f) -> p a f", a=2), in_=M2v[n]
                )
                m2f = stage.tile([128, 2 * HID], F32R, tag="m2f")
                nc.vector.tensor_copy(m2f, m2t)
                nc.vector.tensor_mul(w2t, w2t, m2f)
                W2.append(w2t)

            # W3 per made: load [128, 128] (k-halves side by side), mask, then
            # de-interleave output columns so each k-half block is
            # [shift cols 0:32 | ls cols 32:64].
            W3 = []
            M3v = m3_d.rearrange("n (a p) f -> n p a f", a=2)
            for n in range(NPC):
                w3s = stage.tile([128, 2 * 2 * D], F32R, tag="w3s")
                nc.sync.dma_start(
                    out=w3s.rearrange("p (a f) -> p a f", a=2),
                    in_=w3_d[n].rearrange("(a p) f -> p a f", a=2),
                )
                m3t = stage.tile([128, 2 * 2 * D], U8, tag="m3s")
                nc.sync.dma_start(
                    out=m3t.rearrange("p (a f) -> p a f", a=2), in_=M3v[n]
                )
                m3f = stage.tile([128, 2 * 2 * D], F32R, tag="m3f")
                nc.vector.tensor_copy(m3f, m3t)
                nc.vector.tensor_mul(w3s, w3s, m3f)
                w3r = persist.tile([128, 2 * 2 * D], F32R, tag=f"w3_{n}")
                for h in range(2):
                    blk = w3s[:, 64 * h : 64 * h + 64].rearrange(
                        "p (f two) -> p two f", two=2
                    )
                    nc.vector.tensor_copy(w3r[:, 64 * h : 64 * h + 32], blk[:, 0, :])
                    nc.vector.tensor_copy(
                        w3r[:, 64 * h + 32 : 64 * h + 64], blk[:, 1, :]
                    )
                W3.append(w3r)

            # ---- main compute ------------------------------------------------
            for n in range(NPC):
                g, i = n // 4, n % 4
                rs = slice(32 * i, 32 * i + 32)
                w2t, w3r = W2[n], W3[n]
                for c in range(NCH):
                    cs = slice(c * CH, (c + 1) * CH)

                    ph1a = psum_h.tile([128, CH], F32, tag="ph")
                    nc.tensor.matmul(
                        ph1a,
                        _mm(W1[g][rs, 0:128]),
                        _mm(xrT[rs, cs]),
                        start=True,
                        stop=True,
                        tile_position=(32 * i, 0),
                    )
                    ph1b = psum_h.tile([128, CH], F32, tag="ph")
                    nc.tensor.matmul(
                        ph1b,
                        _mm(W1[g][rs, 128:256]),
                        _mm(xrT[rs, cs]),
                        start=True,
                        stop=True,
                        tile_position=(32 * i, 0),
                    )
                    h1a = work.tile([128, CH], F32R, tag="h1a")
                    nc.scalar.activation(h1a, ph1a, AFT.Relu, bias=zbias)
                    h1b = work.tile([128, CH], F32R, tag="h1b")
                    nc.vector.tensor_scalar_max(h1b, ph1b, 0.0)

                    ph2a = psum_h.tile([128, CH], F32, tag="ph")
                    nc.tensor.matmul(
                        ph2a, _mm(w2t[:, 0:128]), _mm(h1a), start=True, stop=False
                    )
                    nc.tensor.matmul(
                        ph2a,
                        _mm(w2t[:, HID : HID + 128]),
                        _mm(h1b),
                        start=False,
                        stop=True,
                    )
                    ph2b = psum_h.tile([128, CH], F32, tag="ph")
                    nc.tensor.matmul(
                        ph2b, _mm(w2t[:, 128:256]), _mm(h1a), start=True, stop=False
                    )
                    nc.tensor.matmul(
                        ph2b,
                        _mm(w2t[:, HID + 128 : 2 * HID]),
                        _mm(h1b),
                        start=False,
                        stop=True,
                    )
                    h2a = work.tile([128, CH], F32R, tag="h2a")
                    nc.scalar.activation(h2a, ph2a, AFT.Relu, bias=zbias)
                    h2b = work.tile([128, CH], F32R, tag="h2b")
                    nc.vector.tensor_scalar_max(h2b, ph2b, 0.0)

                    po3 = psum_o.tile([64, CH], F32, tag="po")
                    nc.tensor.matmul(
                        po3, _mm(w3r[:, 0:64]), _mm(h2a), start=True, stop=False
                    )
                    nc.tensor.matmul(
                        po3, _mm(w3r[:, 64:128]), _mm(h2b), start=False, stop=True
                    )
                    shift, ls = po3[0:32, :], po3[32:64, :]

                    e = work.tile([32, CH], F32, tag="e")
                    nc.scalar.activation(e, ls, AFT.Exp, bias=zbias[0:32], scale=-1.0)
                    t = work.tile([32, CH], F32, tag="t")
                    nc.vector.tensor_sub(t, xrT[0:32, cs], shift)
                    y = work.tile([32, CH], F32, tag="y")
                    nc.vector.tensor_mul(y, t, e)

                    z = work.tile([64, CH], F32R, tag="z")
                    nc.vector.tensor_mul(z[0:32, :], y, y)
                    # z_ls = ls + C so the coeff matmul folds in the
                    # -D*C normalization constant.
                    nc.scalar.activation(
                        z[32:64, :], ls, AFT.Identity, bias=cbias
                    )

                    pll = psum_l.tile([1, CH], F32, tag="pll")
                    nc.tensor.matmul(pll, _mm(coeff), _mm(z))
                    llt = work.tile([1, CH], F32, tag="llt")
                    nc.scalar.copy(llt, pll)
                    nc.sync.dma_start(out=out_d[n : n + 1, cs], in_=llt)

    nc.compile()
    return nc


_NC_CACHE = None
RUN_KWARGS = {}
LAST_RESULT = None


def _get_nc():
    global _NC_CACHE
    if _NC_CACHE is None:
        _NC_CACHE = build_nc()
    return _NC_CACHE


def kernel(x, w1, w2, w3, m1, m2, m3):
    from concourse.bass_utils import run_bass_kernel_spmd

    x = np.asarray(x, dtype=np.float32)
    w1 = np.asarray(w1, dtype=np.float32)
    w2 = np.asarray(w2, dtype=np.float32)
    w3 = np.asarray(w3, dtype=np.float32)
    m1 = np.asarray(m1).astype(np.uint8)
    m2 = np.asarray(m2).astype(np.uint8)
    m3 = np.asarray(m3).astype(np.uint8)

    xT = np.ascontiguousarray(x[:, :D].T)

    in_maps = []
    for k in range(NCORES):
        s = slice(k * NPC, (k + 1) * NPC)
        in_maps.append(
            {
                "xT": xT,
                "w1": np.ascontiguousarray(w1[s]),
                "w2": np.ascontiguousarray(w2[s]),
                "w3": np.ascontiguousarray(w3[s]),
                "m1": np.ascontiguousarray(m1[s]),
                "m2": np.ascontiguousarray(m2[s]),
                "m3": np.ascontiguousarray(m3[s]),
            }
        )

    nc = _get_nc()
    res = run_bass_kernel_spmd(nc, in_maps, list(range(NCORES)), **RUN_KWARGS)
    global LAST_RESULT
    LAST_RESULT = res
    results = res.results
    # per-core output is ll^T [NPC, B]; assemble to [B, N_BATCH]
    return np.concatenate([results[k]["out"].T for k in range(NCORES)], axis=1)


# revision 12
# speedup vs baseline: 1.3976x; 1.3976x over previous
"""Trainium2 Bass kernel for nn_BatchMAFLayer.

Computes, for a batch of B=4096 inputs and N_BATCH=64 MADE/MAF
distributions, the per-(sample, distribution) log-likelihood:

    xr = x[:, :32]
    h  = relu(xr @ (w1*m1)[n]); h = relu(h @ (w2*m2)[n]); o = h @ (w3*m3)[n]
    shift, ls = o d-major-deinterleaved
    y  = (xr - shift) * exp(-ls)
    ll[b, n] = sum_d(-0.5*y^2 - C - ls)

Sharding: the 64 MADEs are split across 8 NeuronCores (8 per core);
every core sees the full batch and writes its 8 output rows (the host
transposes/concatenates per-core [8, B] blocks into [B, 64]).

On-device layout is feature-major ("T-space"): activations are stored
as [features, batch] so each layer is matmul(W.T @ actT) with the
weight stack used directly as the stationary (lhsT) operand and no
transposes between layers. MADEs are processed in pairs: layer-1
matmuls of a pair sit in different PE row-groups (K=32 row tiling),
layer-3 outputs of the pair col-tile into one PSUM bank (made A ->
partitions 0:64, made B -> 64:128), and the final d-reduction is one
K=128 matmul against a constant coefficient stack.

Matmuls run in bf16 (fp32r caps the PE clock at half rate); the final
reduction matmul and its inputs stay fp32r for summation accuracy, and
the (x - shift) term uses exact f32 x.
"""

import numpy as np

import concourse.bass as bass
from concourse import bacc
import concourse.mybir as mybir
import concourse.tile as tile

F32 = mybir.dt.float32
F32R = mybir.dt.float32r
BF16 = mybir.dt.bfloat16
U8 = mybir.dt.uint8
AFT = mybir.ActivationFunctionType

D = 32
N_BATCH = 64
HID = 256
B = 4096
F = 64
NCORES = 8
NPC = N_BATCH // NCORES  # mades per core
CH = 512                 # batch columns per PSUM tile
NCH = B // CH
HALF_LOG_2PI = 0.5 * float(np.log(2.0 * np.pi))

MM_DT = BF16   # network matmul dtype
LL_DT = F32R   # final-reduction matmul dtype


def build_nc():
    nc = bacc.Bacc("TRN2", target_bir_lowering=False)

    xT_d = nc.dram_tensor("xT", [D, B], F32, kind="ExternalInput")
    w1_d = nc.dram_tensor("w1", [NPC, D, HID], F32, kind="ExternalInput")
    w2_d = nc.dram_tensor("w2", [NPC, HID, HID], F32, kind="ExternalInput")
    w3_d = nc.dram_tensor("w3", [NPC, HID, 2 * D], F32, kind="ExternalInput")
    m1_d = nc.dram_tensor("m1", [NPC, D, HID], U8, kind="ExternalInput")
    m2_d = nc.dram_tensor("m2", [NPC, HID, HID], U8, kind="ExternalInput")
    m3_d = nc.dram_tensor("m3", [NPC, HID, 2 * D], U8, kind="ExternalInput")
    out_d = nc.dram_tensor("out", [NPC, B], F32, kind="ExternalOutput")

    with tile.TileContext(nc) as tc:
        with (
            tc.tile_pool(name="persist", bufs=1) as persist,
            tc.tile_pool(name="stage", bufs=2) as stage,
            tc.tile_pool(name="work", bufs=2) as work,
            tc.tile_pool(name="psum_h", bufs=6, space="PSUM") as psum_h,
            tc.tile_pool(name="psum_o", bufs=1, space="PSUM") as psum_o,
            tc.tile_pool(name="psum_l", bufs=1, space="PSUM") as psum_l,
        ):
            # x[:, :D] transposed: f32 master copy replicated to all four
            # 32-partition groups (row-tiled L1 + pair final stage), plus a
            # bf16 copy for the L1 matmuls.
            xrTf = persist.tile([128, B], F32, tag="xrTf")
            for r in range(4):
                nc.sync.dma_start(out=xrTf[32 * r : 32 * r + 32, :], in_=xT_d[:, :])
            xrT = persist.tile([128, B], MM_DT, tag="xrT")
            nc.vector.tensor_copy(xrT, xrTf)

            # coefficient stack for the pair d-reduction matmul:
            # col 0 reduces made A (rows 0:64), col 1 made B (rows 64:128);
            # rows = [-0.5]*32 (y^2) then [-1.0]*32 (ls + C).
            coeff_f = persist.tile([128, 2], F32, tag="coeff_f")
            nc.vector.memset(coeff_f, 0.0)
            nc.vector.memset(coeff_f[0:32, 0:1], -0.5)
            nc.vector.memset(coeff_f[32:64, 0:1], -1.0)
            nc.vector.memset(coeff_f[64:96, 1:2], -0.5)
            nc.vector.memset(coeff_f[96:128, 1:2], -1.0)
            coeff = persist.tile([128, 2], LL_DT, tag="coeff")
            nc.vector.tensor_copy(coeff, coeff_f)

            # per-partition bias columns for ScalarE activations
            zbias = persist.tile([128, 1], F32, tag="zbias")
            nc.vector.memset(zbias, 0.0)
            cbias = persist.tile([32, 1], F32, tag="cbias")
            nc.vector.memset(cbias, HALF_LOG_2PI)

            # ---- weight prep -------------------------------------------------
            # W1 packed 4 mades per [128, HID] tile (made i at partitions 32i).
            W1 = []
            for g in range(NPC // 4):
                w1s = stage.tile([128, HID], F32, tag="w1s")
                nc.sync.dma_start(
                    out=w1s, in_=w1_d[4 * g : 4 * g + 4].rearrange("a p f -> (a p) f")
                )
                m1t = stage.tile([128, HID], U8, tag="m1s")
                nc.sync.dma_start(
                    out=m1t, in_=m1_d[4 * g : 4 * g + 4].rearrange("a p f -> (a p) f")
                )
                m1f = stage.tile([128, HID], F32, tag="m1f")
                nc.scalar.copy(m1f, m1t)
                w1t = persist.tile([128, HID], MM_DT, tag=f"w1_{g}")
                nc.vector.tensor_mul(w1t, w1s, m1f)
                W1.append(w1t)

            # W2 per made: [128, 2*HID]; cols [0:HID] = rows 0:128 (k-half a),
            # cols [HID:] = rows 128:256 (k-half b).
            W2 = []
            M2v = m2_d.rearrange("n (a p) f -> n p a f", a=2)
            for n in range(NPC):
                w2s = stage.tile([128, 2 * HID], F32, tag="w2s")
                nc.sync.dma_start(
                    out=w2s.rearrange("p (a f) -> p a f", a=2),
                    in_=w2_d[n].rearrange("(a p) f -> p a f", a=2),
                )
                m2t = stage.tile([128, 2 * HID], U8, tag="m2s")
                nc.sync.dma_start(
                    out=m2t.rearrange("p (a f) -> p a f", a=2), in_=M2v[n]
                )
                m2f = stage.tile([128, 2 * HID], F32, tag="m2f")
                nc.scalar.copy(m2f, m2t)
                w2t = persist.tile([128, 2 * HID], MM_DT, tag=f"w2_{n}")
                nc.vector.tensor_mul(w2t, w2s, m2f)
                W2.append(w2t)

            # W3 per made: load [128, 128] (k-halves side by side), mask, then
            # de-interleave output columns so each k-half block is
            # [shift cols 0:32 | ls cols 32:64].
            W3 = []
            M3v = m3_d.rearrange("n (a p) f -> n p a f", a=2)
            for n in range(NPC):
                w3s = stage.tile([128, 2 * 2 * D], F32, tag="w3s")
                nc.sync.dma_start(
                    out=w3s.rearrange("p (a f) -> p a f", a=2),
                    in_=w3_d[n].rearrange("(a p) f -> p a f", a=2),
                )
                m3t = stage.tile([128, 2 * 2 * D], U8, tag="m3s")
                nc.sync.dma_start(
                    out=m3t.rearrange("p (a f) -> p a f", a=2), in_=M3v[n]
                )
                m3f = stage.tile([128, 2 * 2 * D], F32, tag="m3f")
                nc.scalar.copy(m3f, m3t)
                w3m = stage.tile([128, 2 * 2 * D], MM_DT, tag="w3m")
                nc.vector.tensor_mul(w3m, w3s, m3f)
                w3r = persist.tile([128, 2 * 2 * D], MM_DT, tag=f"w3_{n}")
                for h in range(2):
                    blk = w3m[:, 64 * h : 64 * h + 64].rearrange(
                        "p (f two) -> p two f", two=2
                    )
                    nc.vector.tensor_copy(w3r[:, 64 * h : 64 * h + 32], blk[:, 0, :])
                    nc.vector.tensor_copy(
                        w3r[:, 64 * h + 32 : 64 * h + 64], blk[:, 1, :]
                    )
                W3.append(w3r)

            # ---- main compute: mades in pairs --------------------------------
            for j in range(NPC // 2):
                nA, nB = 2 * j, 2 * j + 1
                gA, iA = nA // 4, nA % 4
                gB, iB = nB // 4, nB % 4
                rsA = slice(32 * iA, 32 * iA + 32)
                rsB = slice(32 * iB, 32 * iB + 32)
                for c in range(NCH):
                    cs = slice(c * CH, (c + 1) * CH)

                    # L1: K=32 row-tiled, A and B interleaved for PE-array
                    # concurrency across row groups.
                    ph1 = {}
                    for half, mo in (("a", 0), ("b", 128)):
                        for made, (g, i, rs) in (
                            ("A", (gA, iA, rsA)),
                            ("B", (gB, iB, rsB)),
                        ):
                            p = psum_h.tile([128, CH], F32, tag="ph")
                            nc.tensor.matmul(
                                p,
                                W1[g][rs, mo : mo + 128],
                                xrT[rs, cs],
                                start=True,
                                stop=True,
                                tile_position=(32 * i, 0),
                            )
                            ph1[made + half] = p
                    h1 = {}
                    for k, eng in (
                        ("Aa", "act"), ("Ab", "dve"), ("Ba", "act"), ("Bb", "dve")
                    ):
                        made, half = k[0], k[1]
                        t_ = work.tile([128, CH], MM_DT, tag=f"h1{k}")
                        src = ph1[made + half]
                        if eng == "act":
                            nc.scalar.activation(t_, src, AFT.Relu, bias=zbias)
                        else:
                            nc.vector.tensor_scalar_max(t_, src, 0.0)
                        h1[k] = t_

                    # L2: K=256 in two chunks per output half.
                    ph2 = {}
                    for made, w2t in (("A", W2[nA]), ("B", W2[nB])):
                        for half, mo in (("a", 0), ("b", 128)):
                            p = psum_h.tile([128, CH], F32, tag="ph")
                            nc.tensor.matmul(
                                p,
                                w2t[:, mo : mo + 128],
                                h1[made + "a"],
                                start=True,
                                stop=False,
                            )
                            nc.tensor.matmul(
                                p,
                                w2t[:, HID + mo : HID + mo + 128],
                                h1[made + "b"],
                                start=False,
                                stop=True,
                            )
                            ph2[made + half] = p
                    h2 = {}
                    for k, eng in (
                        ("Aa", "act"), ("Ab", "dve"), ("Ba", "act"), ("Bb", "dve")
                    ):
                        made, half = k[0], k[1]
                        t_ = work.tile([128, CH], MM_DT, tag=f"h2{k}")
                        src = ph2[made + half]
                        if eng == "act":
                            nc.scalar.activation(t_, src, AFT.Relu, bias=zbias)
                        else:
                            nc.vector.tensor_scalar_max(t_, src, 0.0)
                        h2[k] = t_

                    # L3: pair col-tiled into one PSUM bank.
                    # made A -> partitions 0:64, made B -> 64:128.
                    po3 = psum_o.tile([128, CH], F32, tag="po")
                    nc.tensor.matmul(
                        po3[0:64, :], W3[nA][:, 0:64], h2["Aa"],
                        start=True, stop=False, skip_group_check=True,
                    )
                    nc.tensor.matmul(
                        po3[64:128, :], W3[nB][:, 0:64], h2["Ba"],
                        start=True, stop=False, skip_group_check=True,
                    )
                    nc.tensor.matmul(
                        po3[0:64, :], W3[nA][:, 64:128], h2["Ab"],
                        start=False, stop=True, skip_group_check=True,
                    )
                    nc.tensor.matmul(
                        po3[64:128, :], W3[nB][:, 64:128], h2["Bb"],
                        start=False, stop=True, skip_group_check=True,
                    )
                    # rows: 0:32 shift_A, 32:64 ls_A, 64:96 shift_B, 96:128 ls_B

                    # final stage; e/y keep A at rows 0:32, B at rows 64:96
                    # (same -32 partition shift from ls rows for both).
                    e = work.tile([128, CH], F32, tag="e")
                    nc.scalar.activation(
                        e[0:32, :], po3[32:64, :], AFT.Exp,
                        bias=zbias[0:32], scale=-1.0,
                    )
                    nc.scalar.activation(
                        e[64:96, :], po3[96:128, :], AFT.Exp,
                        bias=zbias[0:32], scale=-1.0,
                    )
                    t = work.tile([128, CH], F32, tag="t")
                    nc.vector.tensor_sub(t, xrTf[:, cs], po3)  # rows 32:64/96:128 junk
                    y = work.tile([128, CH], F32, tag="y")
                    nc.vector.tensor_mul(y[0:32, :], t[0:32, :], e[0:32, :])
                    nc.vector.tensor_mul(y[64:96, :], t[64:96, :], e[64:96, :])

                    z = work.tile([128, CH], LL_DT, tag="z")
                    nc.vector.tensor_mul(z[0:32, :], y[0:32, :], y[0:32, :])
                    nc.vector.tensor_mul(z[64:96, :], y[64:96, :], y[64:96, :])
                    # z_ls = ls + C folds the -D*C normalization constant
                    # into the coeff matmul.
                    nc.scalar.activation(
                        z[32:64, :], po3[32:64, :], AFT.Identity, bias=cbias
                    )
                    nc.scalar.activation(
                        z[96:128, :], po3[96:128, :], AFT.Identity, bias=cbias
                    )

                    pll = psum_l.tile([2, CH], F32, tag="pll")
                    nc.tensor.matmul(pll, coeff, z, start=True, stop=True)
                    llt = work.tile([2, CH], F32, tag="llt")
                    nc.scalar.copy(llt, pll)
                    nc.sync.dma_start(out=out_d[nA : nA + 2, cs], in_=llt)

    nc.compile()
    return nc


_NC_CACHE = None
RUN_KWARGS = {}
LAST_RESULT = None


def _get_nc():
    global _NC_CACHE
    if _NC_CACHE is None:
        _NC_CACHE = build_nc()
    return _NC_CACHE


def kernel(x, w1, w2, w3, m1, m2, m3):
    from concourse.bass_utils import run_bass_kernel_spmd

    x = np.asarray(x, dtype=np.float32)
    w1 = np.asarray(w1, dtype=np.float32)
    w2 = np.asarray(w2, dtype=np.float32)
    w3 = np.asarray(w3, dtype=np.float32)
    m1 = np.asarray(m1).astype(np.uint8)
    m2 = np.asarray(m2).astype(np.uint8)
    m3 = np.asarray(m3).astype(np.uint8)

    xT = np.ascontiguousarray(x[:, :D].T)

    in_maps = []
    for k in range(NCORES):
        s = slice(k * NPC, (k + 1) * NPC)
        in_maps.append(
            {
                "xT": xT,
                "w1": np.ascontiguousarray(w1[s]),
                "w2": np.ascontiguousarray(w2[s]),
                "w3": np.ascontiguousarray(w3[s]),
                "m1": np.ascontiguousarray(m1[s]),
                "m2": np.ascontiguousarray(m2[s]),
                "m3": np.ascontiguousarray(m3[s]),
            }
        )

    nc = _get_nc()
    res = run_bass_kernel_spmd(nc, in_maps, list(range(NCORES)), **RUN_KWARGS)
    global LAST_RESULT
    LAST_RESULT = res
    results = res.results
    # per-core output is ll^T [NPC, B]; assemble to [B, N_BATCH]
    return np.concatenate([results[k]["out"].T for k in range(NCORES)], axis=1)


# revision 15
# speedup vs baseline: 1.8309x; 1.3100x over previous
"""Trainium2 Bass kernel for nn_BatchMAFLayer.

Computes, for a batch of B=4096 inputs and N_BATCH=64 MADE/MAF
distributions, the per-(sample, distribution) log-likelihood:

    xr = x[:, :32]
    h  = relu(xr @ (w1*m1)[n]); h = relu(h @ (w2*m2)[n]); o = h @ (w3*m3)[n]
    shift, ls = o d-major-deinterleaved
    y  = (xr - shift) * exp(-ls)
    ll[b, n] = sum_d(-0.5*y^2 - C - ls)

Sharding: the 64 MADEs are split across 8 NeuronCores (8 per core);
every core sees the full batch and writes its 8 output rows (the host
transposes/concatenates per-core [8, B] blocks into [B, 64]).

Device layout is feature-major ("T-space"): activations are stored as
[features, batch] so each layer is matmul(W.T @ actT) with the weight
stack used directly as the stationary (lhsT) operand — no transposes
between layers. Elementwise op time on ACT/DVE scales with the free
(batch) dim only, so ops are merged along partitions wherever possible:

- MADEs are processed in pairs; each hidden-layer PSUM tile is
  [128, 1024] holding both 128-feature halves of one made, consumed by
  a single relu-copy.
- Layer 3 is eight col-tiled M=32 matmuls arranging the pair's outputs
  as [shift_A|shift_B|ls_A|ls_B] so the whole final stage is one op per
  algebraic step (exp / sub / mul / square / +C) on 64-partition rows.
- The d-reduction matmul accumulates all 4 pairs of a chunk into one
  [8, 512] PSUM tile via a [128, 8] coefficient stack with
  zero-padded columns, giving one PSUM->SBUF copy and one output DMA
  per 512-column chunk.

Matmuls run in bf16 (fp32r caps the PE clock at half rate); the final
reduction matmul and its inputs stay fp32r for summation accuracy, and
the (x - shift) term uses exact f32 x.
"""

import numpy as np

import concourse.bass as bass
from concourse import bacc
import concourse.mybir as mybir
import concourse.tile as tile

F32 = mybir.dt.float32
F32R = mybir.dt.float32r
BF16 = mybir.dt.bfloat16
U8 = mybir.dt.uint8
AFT = mybir.ActivationFunctionType

D = 32
N_BATCH = 64
HID = 256
B = 4096
F = 64
NCORES = 8
NPC = N_BATCH // NCORES  # mades per core
CH = 512                 # batch columns per PSUM tile
NCH = B // CH
HALF_LOG_2PI = 0.5 * float(np.log(2.0 * np.pi))

MM_DT = BF16   # network matmul dtype
LL_DT = F32R   # final-reduction matmul dtype


def build_nc():
    nc = bacc.Bacc("TRN2", target_bir_lowering=False)

    xT_d = nc.dram_tensor("xT", [D, B], F32, kind="ExternalInput")
    w1_d = nc.dram_tensor("w1", [NPC, D, HID], F32, kind="ExternalInput")
    w2_d = nc.dram_tensor("w2", [NPC, HID, HID], F32, kind="ExternalInput")
    w3_d = nc.dram_tensor("w3", [NPC, HID, 2 * D], F32, kind="ExternalInput")
    m1_d = nc.dram_tensor("m1", [NPC, D, HID], U8, kind="ExternalInput")
    m2_d = nc.dram_tensor("m2", [NPC, HID, HID], U8, kind="ExternalInput")
    m3_d = nc.dram_tensor("m3", [NPC, HID, 2 * D], U8, kind="ExternalInput")
    out_d = nc.dram_tensor("out", [NPC, B], F32, kind="ExternalOutput")

    with tile.TileContext(nc) as tc:
        with (
            tc.tile_pool(name="persist", bufs=1) as persist,
            tc.tile_pool(name="stage", bufs=2) as stage,
            tc.tile_pool(name="work", bufs=2) as work,
            tc.tile_pool(name="psum_h", bufs=3, space="PSUM") as psum_h,
            tc.tile_pool(name="psum_o", bufs=1, space="PSUM") as psum_o,
            tc.tile_pool(name="psum_l", bufs=1, space="PSUM") as psum_l,
        ):
            # x[:, :D] transposed: f32 master copy replicated to all four
            # 32-partition groups (row-tiled L1 + pair final stage), plus a
            # bf16 copy for the L1 matmuls.
            xrTf = persist.tile([128, B], F32, tag="xrTf")
            for r in range(4):
                nc.sync.dma_start(out=xrTf[32 * r : 32 * r + 32, :], in_=xT_d[:, :])
            xrT = persist.tile([128, B], MM_DT, tag="xrT")
            nc.vector.tensor_copy(xrT, xrTf)

            # coefficient stack for the chunk d-reduction matmul. Column
            # n reduces made n; each pair's z rows are
            # [y2_A | y2_B | ls'_A | ls'_B] (32 rows each), and columns for
            # the other pairs are zero so all four pairs accumulate into one
            # [8, CH] PSUM tile.
            # block j (cols 8j:8j+8) is pair j's stack: only columns 2j and
            # 2j+1 are non-zero, so pair j's matmul contributes solely to
            # output rows 2j:2j+2 while all pairs accumulate one PSUM tile.
            coeff_f = persist.tile([128, NPC * (NPC // 2)], F32, tag="coeff_f")
            nc.vector.memset(coeff_f, 0.0)
            for j in range(NPC // 2):
                for p in range(2):  # made-within-pair (A=0, B=1)
                    col = NPC * j + 2 * j + p
                    nc.vector.memset(coeff_f[32 * p : 32 * p + 32, col : col + 1], -0.5)
                    nc.vector.memset(
                        coeff_f[64 + 32 * p : 96 + 32 * p, col : col + 1], -1.0
                    )
            coeff = persist.tile([128, NPC * (NPC // 2)], LL_DT, tag="coeff")
            nc.vector.tensor_copy(coeff, coeff_f)

            # per-partition bias columns for ScalarE activations
            zbias = persist.tile([128, 1], F32, tag="zbias")
            nc.vector.memset(zbias, 0.0)
            cbias = persist.tile([64, 1], F32, tag="cbias")
            nc.vector.memset(cbias, HALF_LOG_2PI)

            # ---- weight prep -------------------------------------------------
            # W1 packed 4 mades per [128, HID] tile (made i at partitions 32i).
            W1 = []
            for g in range(NPC // 4):
                w1s = stage.tile([128, HID], F32, tag="w1s")
                nc.sync.dma_start(
                    out=w1s, in_=w1_d[4 * g : 4 * g + 4].rearrange("a p f -> (a p) f")
                )
                m1t = stage.tile([128, HID], U8, tag="m1s")
                nc.sync.dma_start(
                    out=m1t, in_=m1_d[4 * g : 4 * g + 4].rearrange("a p f -> (a p) f")
                )
                m1f = stage.tile([128, HID], F32, tag="m1f")
                nc.scalar.copy(m1f, m1t)
                w1t = persist.tile([128, HID], MM_DT, tag=f"w1_{g}")
                nc.vector.tensor_mul(w1t, w1s, m1f)
                W1.append(w1t)

            # W2 per made: [128, 2*HID]; cols [0:HID] = rows 0:128 (k-half a),
            # cols [HID:] = rows 128:256 (k-half b).
            W2 = []
            M2v = m2_d.rearrange("n (a p) f -> n p a f", a=2)
            for n in range(NPC):
                w2s = stage.tile([128, 2 * HID], F32, tag="w2s")
                nc.sync.dma_start(
                    out=w2s.rearrange("p (a f) -> p a f", a=2),
                    in_=w2_d[n].rearrange("(a p) f -> p a f", a=2),
                )
                m2t = stage.tile([128, 2 * HID], U8, tag="m2s")
                nc.sync.dma_start(
                    out=m2t.rearrange("p (a f) -> p a f", a=2), in_=M2v[n]
                )
                m2f = stage.tile([128, 2 * HID], F32, tag="m2f")
                nc.scalar.copy(m2f, m2t)
                w2t = persist.tile([128, 2 * HID], MM_DT, tag=f"w2_{n}")
                nc.vector.tensor_mul(w2t, w2s, m2f)
                W2.append(w2t)

            # W3 per made: load [128, 128] (k-halves side by side), mask, then
            # de-interleave output columns so each k-half block is
            # [shift cols 0:32 | ls cols 32:64].
            W3 = []
            M3v = m3_d.rearrange("n (a p) f -> n p a f", a=2)
            for n in range(NPC):
                w3s = stage.tile([128, 2 * 2 * D], F32, tag="w3s")
                nc.sync.dma_start(
                    out=w3s.rearrange("p (a f) -> p a f", a=2),
                    in_=w3_d[n].rearrange("(a p) f -> p a f", a=2),
                )
                m3t = stage.tile([128, 2 * 2 * D], U8, tag="m3s")
                nc.sync.dma_start(
                    out=m3t.rearrange("p (a f) -> p a f", a=2), in_=M3v[n]
                )
                m3f = stage.tile([128, 2 * 2 * D], F32, tag="m3f")
                nc.scalar.copy(m3f, m3t)
                w3m = stage.tile([128, 2 * 2 * D], MM_DT, tag="w3m")
                nc.vector.tensor_mul(w3m, w3s, m3f)
                w3r = persist.tile([128, 2 * 2 * D], MM_DT, tag=f"w3_{n}")
                for h in range(2):
                    blk = w3m[:, 64 * h : 64 * h + 64].rearrange(
                        "p (f two) -> p two f", two=2
                    )
                    nc.vector.tensor_copy(w3r[:, 64 * h : 64 * h + 32], blk[:, 0, :])
                    nc.vector.tensor_copy(
                        w3r[:, 64 * h + 32 : 64 * h + 64], blk[:, 1, :]
                    )
                W3.append(w3r)

            # ---- main compute: chunk-outer, made-pairs inner ----------------
            for c in range(NCH):
                cs = slice(c * CH, (c + 1) * CH)
                pll = psum_l.tile([NPC, CH], F32, tag="pll")
                for j in range(NPC // 2):
                    nA, nB = 2 * j, 2 * j + 1
                    gA, iA = nA // 4, nA % 4
                    gB, iB = nB // 4, nB % 4
                    rsA = slice(32 * iA, 32 * iA + 32)
                    rsB = slice(32 * iB, 32 * iB + 32)

                    # L1: K=32 row-tiled; per made a [128, 1024] PSUM tile
                    # holds both 128-feature halves ([:, 0:CH]=a, [:, CH:]=b).
                    ph1A = psum_h.tile([128, 2 * CH], F32, tag="ph")
                    ph1B = psum_h.tile([128, 2 * CH], F32, tag="ph")
                    for mo in (0, 128):
                        nc.tensor.matmul(
                            ph1A[:, mo * 4 : mo * 4 + CH],
                            W1[gA][rsA, mo : mo + 128],
                            xrT[rsA, cs],
                            start=True, stop=True,
                            tile_position=(32 * iA, 0),
                        )
                        nc.tensor.matmul(
                            ph1B[:, mo * 4 : mo * 4 + CH],
                            W1[gB][rsB, mo : mo + 128],
                            xrT[rsB, cs],
                            start=True, stop=True,
                            tile_position=(32 * iB, 0),
                        )
                    h1A = work.tile([128, 2 * CH], MM_DT, tag="h1A")
                    nc.scalar.activation(h1A, ph1A, AFT.Relu, bias=zbias)
                    h1B = work.tile([128, 2 * CH], MM_DT, tag="h1B")
                    nc.vector.tensor_scalar_max(h1B, ph1B, 0.0)

                    # L2: K=256 in two chunks per 128-feature output half.
                    ph2A = psum_h.tile([128, 2 * CH], F32, tag="ph")
                    ph2B = psum_h.tile([128, 2 * CH], F32, tag="ph")
                    for made, ph2, h1, w2t in (
                        ("A", ph2A, h1A, W2[nA]),
                        ("B", ph2B, h1B, W2[nB]),
                    ):
                        for half, mo in (("a", 0), ("b", 128)):
                            dst = ph2[:, (mo * 4) : (mo * 4) + CH]
                            nc.tensor.matmul(
                                dst, w2t[:, mo : mo + 128], h1[:, 0:CH],
                                start=True, stop=False,
                            )
                            nc.tensor.matmul(
                                dst, w2t[:, HID + mo : HID + mo + 128],
                                h1[:, CH : 2 * CH],
                                start=False, stop=True,
                            )
                    h2A = work.tile([128, 2 * CH], MM_DT, tag="h2A")
                    nc.scalar.activation(h2A, ph2A, AFT.Relu, bias=zbias)
                    h2B = work.tile([128, 2 * CH], MM_DT, tag="h2B")
                    nc.vector.tensor_scalar_max(h2B, ph2B, 0.0)

                    # L3: eight col-tiled M=32 matmuls; pair output rows are
                    # [shift_A 0:32 | shift_B 32:64 | ls_A 64:96 | ls_B 96:128]
                    po3 = psum_o.tile([128, CH], F32, tag="po")
                    for kh, mo in ((0, 0), (1, 64)):  # k-half, w3r col offset
                        st, sp = kh == 0, kh == 1
                        for made, h2, w3r, cg in (
                            ("A", h2A, W3[nA], 0),
                            ("B", h2B, W3[nB], 32),
                        ):
                            rhs = h2[:, kh * CH : (kh + 1) * CH]
                            nc.tensor.matmul(
                                po3[cg : cg + 32, :],
                                w3r[:, mo : mo + 32],
                                rhs,
                                start=st, stop=sp, skip_group_check=True,
                                tile_position=(0, cg),
                            )
                            nc.tensor.matmul(
                                po3[64 + cg : 96 + cg, :],
                                w3r[:, mo + 32 : mo + 64],
                                rhs,
                                start=st, stop=sp, skip_group_check=True,
                                tile_position=(0, 64 + cg),
                            )

                    # final stage: one op per algebraic step on 64-row blocks
                    e = work.tile([64, CH], F32, tag="e")
                    nc.scalar.activation(
                        e, po3[64:128, :], AFT.Exp, bias=zbias[0:64], scale=-1.0
                    )
                    t = work.tile([64, CH], F32, tag="t")
                    nc.vector.tensor_sub(t, xrTf[0:64, cs], po3[0:64, :])
                    y = work.tile([64, CH], F32, tag="y")
                    nc.vector.tensor_mul(y, t, e)
                    z = work.tile([128, CH], LL_DT, tag="z")
                    nc.scalar.activation(z[0:64, :], y, AFT.Square, bias=zbias[0:64])
                    nc.scalar.activation(
                        z[64:128, :], po3[64:128, :], AFT.Identity, bias=cbias
                    )

                    # d-reduction: accumulate this pair into the chunk's
                    # [NPC, CH] tile via the zero-padded coefficient stack.
                    nc.tensor.matmul(
                        pll, coeff[:, NPC * j : NPC * (j + 1)], z,
                        start=(j == 0), stop=(j == NPC // 2 - 1),
                        skip_group_check=True,
                    )

                llt = work.tile([NPC, CH], F32, tag="llt")
                nc.scalar.copy(llt, pll)
                nc.sync.dma_start(out=out_d[:, cs], in_=llt)

    nc.compile()
    return nc


_NC_CACHE = None
RUN_KWARGS = {}
LAST_RESULT = None


def _get_nc():
    global _NC_CACHE
    if _NC_CACHE is None:
        _NC_CACHE = build_nc()
    return _NC_CACHE


def kernel(x, w1, w2, w3, m1, m2, m3):
    from concourse.bass_utils import run_bass_kernel_spmd

    x = np.asarray(x, dtype=np.float32)
    w1 = np.asarray(w1, dtype=np.float32)
    w2 = np.asarray(w2, dtype=np.float32)
    w3 = np.asarray(w3, dtype=np.float32)
    m1 = np.asarray(m1).astype(np.uint8)
    m2 = np.asarray(m2).astype(np.uint8)
    m3 = np.asarray(m3).astype(np.uint8)

    xT = np.ascontiguousarray(x[:, :D].T)

    in_maps = []
    for k in range(NCORES):
        s = slice(k * NPC, (k + 1) * NPC)
        in_maps.append(
            {
                "xT": xT,
                "w1": np.ascontiguousarray(w1[s]),
                "w2": np.ascontiguousarray(w2[s]),
                "w3": np.ascontiguousarray(w3[s]),
                "m1": np.ascontiguousarray(m1[s]),
                "m2": np.ascontiguousarray(m2[s]),
                "m3": np.ascontiguousarray(m3[s]),
            }
        )

    nc = _get_nc()
    res = run_bass_kernel_spmd(nc, in_maps, list(range(NCORES)), **RUN_KWARGS)
    global LAST_RESULT
    LAST_RESULT = res
    results = res.results
    # per-core output is ll^T [NPC, B]; assemble to [B, N_BATCH]
    return np.concatenate([results[k]["out"].T for k in range(NCORES)], axis=1)
